# revision 4
# baseline (speedup 1.0000x reference)
"""Expert-parallel MoE FFN kernel for Trainium2 (8 NeuronCores).

Problem: inputs [4, 8192, 1024], per-expert FFN with E=8 experts:
  x -> x @ w1[e].T + b1[e] -> gelu -> @ w2[e].T + b2[e]
Sharding: expert-parallel, one expert per core (DeepSpeed expert-parallel
layout). No collectives needed: core e gets inputs[:, e*C:(e+1)*C, :] and
expert e's weights, produces that slice of the output.

Per-core compute: [4096,1024] @ [1024,4096] -> gelu -> @ [4096,1024]
(68.7 GFLOP). Matmul operands are fp16 (PSUM accumulation stays fp32):
full PE rate with the LDWEIGHTS hidden behind the moving stream. The
4096 N=512 matmuls stream at the 512/2.4GHz+2.5ns = 215.8ns floor
(trace-verified); fp8 DoubleRow was evaluated and rejected: e4m3
rounding gives 3.5e-2..6e-2 rel err vs the 2e-2 gate.

v2 edge-trimming on top of the 97%-of-floor baseline (913.0us):
  - 10 warmup matmuls on a memset tile issued at t=0 so the PE HAM
    clock-gate (4/8 -> 8/8 after ~3.4us of activity) warms during the
    initial DMA wait instead of during the real stream.
  - x tiles loaded round-robin across gpsimd/vector/scalar DMA queues
    (was: all 8 serialized on gpsimd) so the first fc-group's operands
    land ~2x sooner.
  - y stored as fp16 in per-(dc,th) [128,512] chunks alternating
    gpsimd/vector queues (was: fp32 [128,1024] chunks on scalar only):
    the post-last-matmul store drain shrinks ~6us -> ~1us and write
    traffic halves. Host upcasts.

Device layout (all transposes + tiling done host-side, free):
  phase 1: hT[f, t] = gelu(w1T[d, f].T @ xT[d, t] + b1[f])   (K=d on partitions)
  phase 2: yT[d, t] = w2T[f, d].T @ hT[f, t] + b2[d]          (K=f on partitions)
Host untransposes yT -> y. DRAM tensors are pre-packed so every SBUF
tile fills with a single contiguous dma_start.
"""

import time

import numpy as np

import concourse.bacc as bacc
import concourse.mybir as mybir
import concourse.tile as tile
from concourse.bass_utils import run_bass_kernel_spmd
from concourse.mybir import ActivationFunctionType as AFT

E = 8          # experts == cores
D = 1024       # d_model
F = 4096       # d_ff
B, C = 4, 1024
T = B * C      # tokens per expert (4096)
TT = 1024      # token tile
NTT = T // TT  # 4
FBW = 1024     # f-block width
NFB = F // FBW # 4
KD = D // 128  # 8 k-chunks over d
KF = FBW // 128  # 8 f-chunks per f-block
ND = D // 128  # 8 d-chunks
NWARM = 10     # HAM warmup matmuls
f32 = mybir.dt.float32
f16 = mybir.dt.float16

_COMPILED = None  # (nc, input_names)


def _build():
    nc = bacc.Bacc("TRN2", target_bir_lowering=False, debug=False)

    xt_d = nc.dram_tensor("xt", [NTT, KD, 128, TT], f16, kind="ExternalInput")
    w1_d = nc.dram_tensor("w1", [F // 128, 128, KD * 128], f16, kind="ExternalInput")
    w2_d = nc.dram_tensor("w2", [NFB, ND, 128, KF * 128], f16, kind="ExternalInput")
    b1_d = nc.dram_tensor("b1", [128, F // 128], f32, kind="ExternalInput")
    b2_d = nc.dram_tensor("b2", [128, ND], f32, kind="ExternalInput")
    yt_d = nc.dram_tensor("yt", [D, T], f16, kind="ExternalOutput")

    xt = xt_d.ap()
    w1 = w1_d.ap()
    w2 = w2_d.ap()
    yt = yt_d.ap()

    with tile.TileContext(nc) as tc:
        with (
            tc.tile_pool(name="xp", bufs=3) as xp,
            tc.tile_pool(name="w1p", bufs=8) as w1p,
            tc.tile_pool(name="w2p", bufs=8) as w2p,
            tc.tile_pool(name="hp", bufs=2) as hp,
            tc.tile_pool(name="yp", bufs=2) as yp,
            tc.tile_pool(name="yfp", bufs=4) as yfp,
            tc.tile_pool(name="bp", bufs=1) as bp,
            tc.tile_pool(name="hpp", bufs=2, space="PSUM") as hpp,
            tc.tile_pool(name="ypp", bufs=4, space="PSUM") as ypp,
        ):
            # HAM warmup: keep the PE busy from the very start so the
            # clock gate opens to 8/8 before the real stream begins.
            # Reuses a yps PSUM buffer; source tile is memset on vector.
            warm_sb = bp.tile([128, 512], f16, tag="warm")
            nc.vector.memset(warm_sb[:], 0.0)
            warm_ps = ypp.tile([128, 512], f32, tag="yps", name="warm_ps")
            for _ in range(NWARM):
                nc.tensor.matmul(
                    warm_ps[:], warm_sb[:, 0:128], warm_sb[:],
                    start=True, stop=True,
                )

            b1_sb = bp.tile([128, F // 128], f32, tag="b1")
            b2_sb = bp.tile([128, ND], f32, tag="b2")

            # x-load queue rotation: gpsimd (SWDGE) + scalar (HWDGE);
            # sync stays dedicated to the weight stream, vector has no DGE.
            xq = [nc.gpsimd, nc.scalar]

            for tt in range(NTT):
                xks = []
                for k in range(KD):
                    xk = xp.tile([128, TT], f16, tag=f"xk{k}", name=f"xk_{k}")
                    xq[k % 2].dma_start(xk[:], xt[tt, k])
                    xks.append(xk)
                if tt == 0:
                    # issue after the first x tiles so they win the queue
                    nc.scalar.dma_start(b1_sb[:], b1_d.ap()[:])
                    nc.scalar.dma_start(b2_sb[:], b2_d.ap()[:])
                yacc = yp.tile([128, ND * TT], f32, tag="yacc")

                for fb in range(NFB):
                    htile = hp.tile([128, KF * TT], f16, tag="h")
                    # ---- phase 1: hT[fb] = gelu(w1T.T @ xT + b1) ----
                    for fc in range(KF):
                        g = fb * KF + fc
                        w1t = w1p.tile([128, KD * 128], f16, tag="w1")
                        nc.sync.dma_start(w1t[:], w1[g])
                        ph = hpp.tile([128, TT], f32, tag="hps")
                        for k in range(KD):
                            for th in range(TT // 512):
                                nc.tensor.matmul(
                                    ph[:, th * 512:(th + 1) * 512],
                                    w1t[:, k * 128:(k + 1) * 128],
                                    xks[k][:, th * 512:(th + 1) * 512],
                                    start=(k == 0),
                                    stop=(k == KD - 1),
                                )
                        nc.scalar.activation(
                            htile[:, fc * TT:(fc + 1) * TT], ph[:],
                            AFT.Gelu, bias=b1_sb[:, g:g + 1],
                        )

                    # ---- phase 2: yT += w2T.T @ hT[fb] (+ b2 on first block) ----
                    for dcg in range(ND // 2):
                        w2ts = []
                        for j in range(2):
                            dc = dcg * 2 + j
                            w2t = w2p.tile([128, KF * 128], f16, tag="w2")
                            nc.sync.dma_start(w2t[:], w2[fb, dc])
                            w2ts.append(w2t)
                        pys = [
                            ypp.tile([128, 512], f32, tag="yps", name=f"yps_{i}")
                            for i in range(2 * (TT // 512))
                        ]
                        for fc in range(KF):
                            for j in range(2):
                                for th in range(TT // 512):
                                    nc.tensor.matmul(
                                        pys[j * (TT // 512) + th][:],
                                        w2ts[j][:, fc * 128:(fc + 1) * 128],
                                        htile[:, fc * TT + th * 512:fc * TT + (th + 1) * 512],
                                        start=(fc == 0),
                                        stop=(fc == KF - 1),
                                    )  # j-major keeps w2 stationary across th

                        for j in range(2):
                            dc = dcg * 2 + j
                            for th in range(TT // 512):
                                py = pys[j * (TT // 512) + th][:]
                                yc = yacc[:, dc * TT + th * 512:dc * TT + (th + 1) * 512]
                                if fb == 0:
                                    nc.scalar.activation(
                                        yc, py, AFT.Identity, bias=b2_sb[:, dc:dc + 1]
                                    )
                                elif fb < NFB - 1:
                                    nc.vector.tensor_add(yc, yc, py)
                                else:
                                    # final value: add into an fp16 staging
                                    # tile and store this 512-token chunk
                                    # immediately so the writeback overlaps
                                    # the remaining compute and the end-of-
                                    # kernel drain stays ~1 chunk deep.
                                    yf = yfp.tile([128, 512], f16, tag="yf16")
                                    nc.vector.tensor_add(yf[:], yc, py)
                                    qeng = nc.gpsimd if th == 0 else nc.scalar
                                    qeng.dma_start(
                                        yt[dc * 128:(dc + 1) * 128,
                                           tt * TT + th * 512:tt * TT + (th + 1) * 512],
                                        yf[:],
                                    )

    nc.compile()
    return nc


def _get_compiled():
    global _COMPILED
    if _COMPILED is None:
        _COMPILED = _build()
    return _COMPILED


def _pack_core(x_e, w1_e, b1_e, w2_e, b2_e):
    """Host-side repack of one expert's tensors into the kernel's tiled layouts."""
    xT = x_e.reshape(T, D).T                      # [D, T]
    xt = np.ascontiguousarray(
        xT.reshape(KD, 128, NTT, TT).transpose(2, 0, 1, 3)
    ).astype(np.float16)                          # [NTT, KD, 128, TT]
    w1T = w1_e.T                                  # [D, F]
    w1t = np.ascontiguousarray(
        w1T.reshape(KD, 128, F // 128, 128).transpose(2, 1, 0, 3).reshape(F // 128, 128, KD * 128)
    ).astype(np.float16)                          # [F//128, 128, KD*128]
    w2T = w2_e.T                                  # [F, D]
    w2t = np.ascontiguousarray(
        w2T.reshape(NFB, KF, 128, ND, 128).transpose(0, 3, 2, 1, 4).reshape(NFB, ND, 128, KF * 128)
    ).astype(np.float16)                          # [NFB, ND, 128, KF*128]
    b1t = np.ascontiguousarray(b1_e.reshape(F // 128, 128).T)  # [128, F//128]
    b2t = np.ascontiguousarray(b2_e.reshape(ND, 128).T)        # [128, ND]
    return {"xt": xt, "w1": w1t, "w2": w2t, "b1": b1t, "b2": b2t}


def kernel(inputs, w1, b1, w2, b2):
    inputs = np.asarray(inputs, dtype=np.float32)
    w1 = np.asarray(w1, dtype=np.float32)
    b1 = np.asarray(b1, dtype=np.float32)
    w2 = np.asarray(w2, dtype=np.float32)
    b2 = np.asarray(b2, dtype=np.float32)

    nc = _get_compiled()

    in_maps = []
    for e in range(E):
        x_e = inputs[:, e * C:(e + 1) * C, :]     # [B, C, D]
        in_maps.append(_pack_core(x_e, w1[e], b1[e], w2[e], b2[e]))

    # The axon-tunneled devices occasionally come up wedged
    # (NRT_EXEC_UNIT_UNRECOVERABLE on the first execute); a retry after a
    # short pause reliably recovers.
    last_err = None
    for attempt in range(3):
        try:
            res = run_bass_kernel_spmd(nc, in_maps, core_ids=list(range(E)))
            out = np.empty((B, E * C, D), dtype=np.float32)
            for e in range(E):
                yT = np.asarray(res.results[e]["yt"]).astype(np.float32)  # [D, T]
                out[:, e * C:(e + 1) * C, :] = yT.T.reshape(B, C, D)
            return out
        except Exception as err:  # noqa: BLE001 - device flake, retry
            last_err = err
            time.sleep(10 * (attempt + 1))
    raise last_err


# revision 7
# speedup vs baseline: 1.0002x; 1.0002x over previous
"""Expert-parallel MoE FFN kernel for Trainium2 (8 NeuronCores).

Problem: inputs [4, 8192, 1024], per-expert FFN with E=8 experts:
  x -> x @ w1[e].T + b1[e] -> gelu -> @ w2[e].T + b2[e]
Sharding: expert-parallel, one expert per core (DeepSpeed expert-parallel
layout). No collectives needed: core e gets inputs[:, e*C:(e+1)*C, :] and
expert e's weights, produces that slice of the output.

Per-core compute: [4096,1024] @ [1024,4096] -> gelu -> @ [4096,1024]
(68.7 GFLOP). Matmul operands are fp16 (PSUM accumulation stays fp32):
full PE rate with the LDWEIGHTS hidden behind the moving stream. The
4096 N=512 matmuls stream at the 512/2.4GHz+2.5ns = 215.8ns floor
(trace-verified); fp8 DoubleRow was evaluated and rejected: e4m3
rounding gives 3.5e-2..6e-2 rel err vs the 2e-2 gate.

v2 edge-trimming on top of the 97%-of-floor baseline (913.0us):
  - 10 warmup matmuls on a memset tile issued at t=0 so the PE HAM
    clock-gate (4/8 -> 8/8 after ~3.4us of activity) warms during the
    initial DMA wait instead of during the real stream.
  - x tiles loaded round-robin across gpsimd/vector/scalar DMA queues
    (was: all 8 serialized on gpsimd) so the first fc-group's operands
    land ~2x sooner.
  - y stored as fp16 in per-(dc,th) [128,512] chunks alternating
    gpsimd/vector queues (was: fp32 [128,1024] chunks on scalar only):
    the post-last-matmul store drain shrinks ~6us -> ~1us and write
    traffic halves. Host upcasts.

Device layout (all transposes + tiling done host-side, free):
  phase 1: hT[f, t] = gelu(w1T[d, f].T @ xT[d, t] + b1[f])   (K=d on partitions)
  phase 2: yT[d, t] = w2T[f, d].T @ hT[f, t] + b2[d]          (K=f on partitions)
Host untransposes yT -> y. DRAM tensors are pre-packed so every SBUF
tile fills with a single contiguous dma_start.
"""

import time

import numpy as np

import concourse.bacc as bacc
import concourse.mybir as mybir
import concourse.tile as tile
from concourse.bass_utils import run_bass_kernel_spmd
from concourse.mybir import ActivationFunctionType as AFT

E = 8          # experts == cores
D = 1024       # d_model
F = 4096       # d_ff
B, C = 4, 1024
T = B * C      # tokens per expert (4096)
TT = 1024      # token tile
NTT = T // TT  # 4
FBW = 1024     # f-block width
NFB = F // FBW # 4
KD = D // 128  # 8 k-chunks over d
KF = FBW // 128  # 8 f-chunks per f-block
ND = D // 128  # 8 d-chunks
NWARM = 10     # HAM warmup matmuls
f32 = mybir.dt.float32
f16 = mybir.dt.float16

_COMPILED = None  # (nc, input_names)


def _build():
    nc = bacc.Bacc("TRN2", target_bir_lowering=False, debug=False)

    xt_d = nc.dram_tensor("xt", [NTT, KD, 128, TT], f16, kind="ExternalInput")
    w1_d = nc.dram_tensor("w1", [F // 128, 128, KD * 128], f16, kind="ExternalInput")
    w2_d = nc.dram_tensor("w2", [NFB, ND, 128, KF * 128], f16, kind="ExternalInput")
    b1_d = nc.dram_tensor("b1", [128, F // 128], f32, kind="ExternalInput")
    b2_d = nc.dram_tensor("b2", [128, ND], f32, kind="ExternalInput")
    yt_d = nc.dram_tensor("yt", [D, T], f16, kind="ExternalOutput")

    xt = xt_d.ap()
    w1 = w1_d.ap()
    w2 = w2_d.ap()
    yt = yt_d.ap()

    with tile.TileContext(nc) as tc:
        with (
            tc.tile_pool(name="xp", bufs=3) as xp,
            tc.tile_pool(name="w1p", bufs=8) as w1p,
            tc.tile_pool(name="w2p", bufs=8) as w2p,
            tc.tile_pool(name="hp", bufs=2) as hp,
            tc.tile_pool(name="yp", bufs=2) as yp,
            tc.tile_pool(name="yfp", bufs=4) as yfp,
            tc.tile_pool(name="bp", bufs=1) as bp,
            tc.tile_pool(name="hpp", bufs=2, space="PSUM") as hpp,
            tc.tile_pool(name="ypp", bufs=4, space="PSUM") as ypp,
        ):
            # HAM warmup: keep the PE busy from the very start so the
            # clock gate opens to 8/8 before the real stream begins.
            # Reuses a yps PSUM buffer; source tile is memset on vector.
            warm_sb = bp.tile([128, 512], f16, tag="warm")
            nc.vector.memset(warm_sb[:], 0.0)
            warm_ps = ypp.tile([128, 512], f32, tag="yps", name="warm_ps")
            for _ in range(NWARM):
                nc.tensor.matmul(
                    warm_ps[:], warm_sb[:, 0:128], warm_sb[:],
                    start=True, stop=True,
                )

            b1_sb = bp.tile([128, F // 128], f32, tag="b1")
            nc.sync.dma_start(b1_sb[:], b1_d.ap()[:])
            b2_sb = bp.tile([128, ND], f32, tag="b2")
            nc.sync.dma_start(b2_sb[:], b2_d.ap()[:])

            # All loads round-robin over the three DGE-capable issue queues
            # in emission (== consumption) order, so no FIFO queue ever
            # head-blocks a tile needed now behind one needed later, and the
            # aggregate HBM bandwidth always serves the front of the need
            # curve. Stores rotate over gpsimd/sync so the final chunks
            # drain on two queues in parallel.
            loadq = [nc.gpsimd, nc.scalar, nc.sync]
            storeq = [nc.gpsimd, nc.sync]
            qstate = [0, 0]

            def load(dst, src):
                loadq[qstate[0] % 3].dma_start(dst, src)
                qstate[0] += 1

            def store(dst, src):
                storeq[qstate[1] % 2].dma_start(dst, src)
                qstate[1] += 1

            def load_x(tt):
                xks = []
                for k in range(KD):
                    xk = xp.tile([128, TT], f16, tag=f"xk{k}", name=f"xk_{k}")
                    load(xk[:], xt[tt, k])
                    xks.append(xk)
                return xks

            xks_next = load_x(0)
            for tt in range(NTT):
                xks = xks_next
                yacc = yp.tile([128, ND * TT], f32, tag="yacc")

                for fb in range(NFB):
                    if fb == NFB - 2 and tt + 1 < NTT:
                        # prefetch next token tile ~2 f-blocks (~55us) early
                        xks_next = load_x(tt + 1)
                    htile = hp.tile([128, KF * TT], f16, tag="h")
                    # ---- phase 1: hT[fb] = gelu(w1T.T @ xT + b1) ----
                    for fc in range(KF):
                        g = fb * KF + fc
                        w1t = w1p.tile([128, KD * 128], f16, tag="w1")
                        load(w1t[:], w1[g])
                        ph = hpp.tile([128, TT], f32, tag="hps")
                        for k in range(KD):
                            for th in range(TT // 512):
                                nc.tensor.matmul(
                                    ph[:, th * 512:(th + 1) * 512],
                                    w1t[:, k * 128:(k + 1) * 128],
                                    xks[k][:, th * 512:(th + 1) * 512],
                                    start=(k == 0),
                                    stop=(k == KD - 1),
                                )
                        nc.scalar.activation(
                            htile[:, fc * TT:(fc + 1) * TT], ph[:],
                            AFT.Gelu, bias=b1_sb[:, g:g + 1],
                        )

                    # ---- phase 2: yT += w2T.T @ hT[fb] (+ b2 on first block) ----
                    for dcg in range(ND // 2):
                        w2ts = []
                        for j in range(2):
                            dc = dcg * 2 + j
                            w2t = w2p.tile([128, KF * 128], f16, tag="w2")
                            load(w2t[:], w2[fb, dc])
                            w2ts.append(w2t)
                        pys = [
                            ypp.tile([128, 512], f32, tag="yps", name=f"yps_{i}")
                            for i in range(2 * (TT // 512))
                        ]
                        for fc in range(KF):
                            for j in range(2):
                                for th in range(TT // 512):
                                    nc.tensor.matmul(
                                        pys[j * (TT // 512) + th][:],
                                        w2ts[j][:, fc * 128:(fc + 1) * 128],
                                        htile[:, fc * TT + th * 512:fc * TT + (th + 1) * 512],
                                        start=(fc == 0),
                                        stop=(fc == KF - 1),
                                    )  # j-major keeps w2 stationary across th

                        for j in range(2):
                            dc = dcg * 2 + j
                            for th in range(TT // 512):
                                py = pys[j * (TT // 512) + th][:]
                                yc = yacc[:, dc * TT + th * 512:dc * TT + (th + 1) * 512]
                                if fb == 0:
                                    nc.scalar.activation(
                                        yc, py, AFT.Identity, bias=b2_sb[:, dc:dc + 1]
                                    )
                                elif fb < NFB - 1:
                                    nc.vector.tensor_add(yc, yc, py)
                                else:
                                    # final value: add into an fp16 staging
                                    # tile and store this 512-token chunk
                                    # immediately so the writeback overlaps
                                    # the remaining compute and the end-of-
                                    # kernel drain stays ~1 chunk deep.
                                    yf = yfp.tile([128, 512], f16, tag="yf16")
                                    nc.vector.tensor_add(yf[:], yc, py)
                                    store(
                                        yt[dc * 128:(dc + 1) * 128,
                                           tt * TT + th * 512:tt * TT + (th + 1) * 512],
                                        yf[:],
                                    )

    nc.compile()
    return nc


def _get_compiled():
    global _COMPILED
    if _COMPILED is None:
        _COMPILED = _build()
    return _COMPILED


def _pack_core(x_e, w1_e, b1_e, w2_e, b2_e):
    """Host-side repack of one expert's tensors into the kernel's tiled layouts."""
    xT = x_e.reshape(T, D).T                      # [D, T]
    xt = np.ascontiguousarray(
        xT.reshape(KD, 128, NTT, TT).transpose(2, 0, 1, 3)
    ).astype(np.float16)                          # [NTT, KD, 128, TT]
    w1T = w1_e.T                                  # [D, F]
    w1t = np.ascontiguousarray(
        w1T.reshape(KD, 128, F // 128, 128).transpose(2, 1, 0, 3).reshape(F // 128, 128, KD * 128)
    ).astype(np.float16)                          # [F//128, 128, KD*128]
    w2T = w2_e.T                                  # [F, D]
    w2t = np.ascontiguousarray(
        w2T.reshape(NFB, KF, 128, ND, 128).transpose(0, 3, 2, 1, 4).reshape(NFB, ND, 128, KF * 128)
    ).astype(np.float16)                          # [NFB, ND, 128, KF*128]
    b1t = np.ascontiguousarray(b1_e.reshape(F // 128, 128).T)  # [128, F//128]
    b2t = np.ascontiguousarray(b2_e.reshape(ND, 128).T)        # [128, ND]
    return {"xt": xt, "w1": w1t, "w2": w2t, "b1": b1t, "b2": b2t}


def kernel(inputs, w1, b1, w2, b2):
    inputs = np.asarray(inputs, dtype=np.float32)
    w1 = np.asarray(w1, dtype=np.float32)
    b1 = np.asarray(b1, dtype=np.float32)
    w2 = np.asarray(w2, dtype=np.float32)
    b2 = np.asarray(b2, dtype=np.float32)

    nc = _get_compiled()

    in_maps = []
    for e in range(E):
        x_e = inputs[:, e * C:(e + 1) * C, :]     # [B, C, D]
        in_maps.append(_pack_core(x_e, w1[e], b1[e], w2[e], b2[e]))

    # The axon-tunneled devices occasionally come up wedged
    # (NRT_EXEC_UNIT_UNRECOVERABLE on the first execute); a retry after a
    # short pause reliably recovers.
    last_err = None
    for attempt in range(3):
        try:
            res = run_bass_kernel_spmd(nc, in_maps, core_ids=list(range(E)))
            out = np.empty((B, E * C, D), dtype=np.float32)
            for e in range(E):
                yT = np.asarray(res.results[e]["yt"]).astype(np.float32)  # [D, T]
                out[:, e * C:(e + 1) * C, :] = yT.T.reshape(B, C, D)
            return out
        except Exception as err:  # noqa: BLE001 - device flake, retry
            last_err = err
            time.sleep(10 * (attempt + 1))
    raise last_err


# revision 9
# speedup vs baseline: 1.0116x; 1.0115x over previous
"""Expert-parallel MoE FFN kernel for Trainium2 (8 NeuronCores).

Problem: inputs [4, 8192, 1024], per-expert FFN with E=8 experts:
  x -> x @ w1[e].T + b1[e] -> gelu -> @ w2[e].T + b2[e]
Sharding: expert-parallel, one expert per core (DeepSpeed expert-parallel
layout). No collectives needed: core e gets inputs[:, e*C:(e+1)*C, :] and
expert e's weights, produces that slice of the output.

Per-core compute: [4096,1024] @ [1024,4096] -> gelu -> @ [4096,1024]
(68.7 GFLOP). Matmul operands are fp16 (PSUM accumulation stays fp32):
full PE rate with the LDWEIGHTS hidden behind the moving stream. The
4096 N=512 matmuls stream at the 512/2.4GHz+2.5ns = 215.8ns floor
(trace-verified, median inter-matmul gap 216ns); fp8 DoubleRow was
evaluated and rejected: e4m3 rounding gives 3.5e-2..6e-2 rel err vs
the 2e-2 gate.

DMA queueing (trace-derived): the Tile framework allows only 8
outstanding HWDGE transfers (shared by sync+scalar issues) plus 8
SWDGE (gpsimd), and a DMA issue whose ring slot is still busy stalls
that whole engine queue. So the queue split is load-class based:
x tiles on gpsimd (SW lanes), the weight stream on sync (HW lanes),
biases + y stores on scalar (HW lanes, interleaving with sync's
weights only every 8th slot). Spreading loads round-robin across
queues was tried and is ~9us slower (couples the latency-critical
first tiles to bulk-weight ring slots).

On top of that baseline (913.0us):
  - 10 warmup matmuls on a memset tile keep the PE busy from ~7.3us so
    the HAM clock gate (cold 1.2GHz -> warm 2.4GHz after ~3.4us of
    sustained activity) opens during the initial DMA wait; the real
    stream then starts warm instead of paying ~4.6us of cold/ramp
    excess.
  - y is stored as fp16 (host upcasts): halves the writeback bytes and
    the post-last-matmul store drain. The final add of each d-chunk
    writes a separate fp16 staging tile which is stored immediately.

Device layout (all transposes + tiling done host-side, free):
  phase 1: hT[f, t] = gelu(w1T[d, f].T @ xT[d, t] + b1[f])   (K=d on partitions)
  phase 2: yT[d, t] = w2T[f, d].T @ hT[f, t] + b2[d]          (K=f on partitions)
Host untransposes yT -> y. DRAM tensors are pre-packed so every SBUF
tile fills with a single contiguous dma_start.
"""

import time

import numpy as np

import concourse.bacc as bacc
import concourse.mybir as mybir
import concourse.tile as tile
from concourse.bass_utils import run_bass_kernel_spmd
from concourse.mybir import ActivationFunctionType as AFT

E = 8          # experts == cores
D = 1024       # d_model
F = 4096       # d_ff
B, C = 4, 1024
T = B * C      # tokens per expert (4096)
TT = 1024      # token tile
NTT = T // TT  # 4
FBW = 1024     # f-block width
NFB = F // FBW # 4
KD = D // 128  # 8 k-chunks over d
KF = FBW // 128  # 8 f-chunks per f-block
ND = D // 128  # 8 d-chunks
NWARM = 10     # HAM warmup matmuls
f32 = mybir.dt.float32
f16 = mybir.dt.float16

_COMPILED = None


def _build():
    nc = bacc.Bacc("TRN2", target_bir_lowering=False, debug=False)

    xt_d = nc.dram_tensor("xt", [NTT, KD, 128, TT], f16, kind="ExternalInput")
    w1_d = nc.dram_tensor("w1", [F // 128, 128, KD * 128], f16, kind="ExternalInput")
    w2_d = nc.dram_tensor("w2", [NFB, ND, 128, KF * 128], f16, kind="ExternalInput")
    b1_d = nc.dram_tensor("b1", [128, F // 128], f32, kind="ExternalInput")
    b2_d = nc.dram_tensor("b2", [128, ND], f32, kind="ExternalInput")
    yt_d = nc.dram_tensor("yt", [D, T], f16, kind="ExternalOutput")

    xt = xt_d.ap()
    w1 = w1_d.ap()
    w2 = w2_d.ap()
    yt = yt_d.ap()

    with tile.TileContext(nc) as tc:
        with (
            tc.tile_pool(name="xp", bufs=3) as xp,
            tc.tile_pool(name="w1p", bufs=8) as w1p,
            tc.tile_pool(name="w2p", bufs=8) as w2p,
            tc.tile_pool(name="hp", bufs=2) as hp,
            tc.tile_pool(name="yp", bufs=2) as yp,
            tc.tile_pool(name="yfp", bufs=4) as yfp,
            tc.tile_pool(name="bp", bufs=1) as bp,
            tc.tile_pool(name="hpp", bufs=2, space="PSUM") as hpp,
            tc.tile_pool(name="ypp", bufs=4, space="PSUM") as ypp,
        ):
            # HAM warmup: PE busy from the start; reuses a yps PSUM buffer.
            warm_sb = bp.tile([128, 512], f16, tag="warm")
            nc.vector.memset(warm_sb[:], 0.0)
            warm_ps = ypp.tile([128, 512], f32, tag="yps", name="warm_ps")
            for _ in range(NWARM):
                nc.tensor.matmul(
                    warm_ps[:], warm_sb[:, 0:128], warm_sb[:],
                    start=True, stop=True,
                )

            b1_sb = bp.tile([128, F // 128], f32, tag="b1")
            nc.scalar.dma_start(b1_sb[:], b1_d.ap()[:])
            b2_sb = bp.tile([128, ND], f32, tag="b2")
            nc.scalar.dma_start(b2_sb[:], b2_d.ap()[:])

            for tt in range(NTT):
                # tt=0 startup: spread the first token tile over all three
                # DGE queues so its 2MB lands ~2x sooner, with w1[0] hoisted
                # between the sync-queue x tiles. The 8 HW-lane ring slots
                # of the first cycle are exactly b1,b2,xk1,xk5 (scalar) +
                # xk3,w1fc0,xk7,w1fc1 (sync), so nothing ring-stalls.
                # tt>0 keeps the steady-state layout (x on gpsimd only).
                xks = []
                w1t_first = None
                for k in range(KD):
                    xk = xp.tile([128, TT], f16, tag=f"xk{k}", name=f"xk_{k}")
                    if tt == 0 and k % 4 == 1:
                        nc.scalar.dma_start(xk[:], xt[tt, k])
                    elif tt == 0 and k % 4 == 3:
                        nc.sync.dma_start(xk[:], xt[tt, k])
                    else:
                        nc.gpsimd.dma_start(xk[:], xt[tt, k])
                    xks.append(xk)
                    if tt == 0 and k == 3:
                        w1t_first = w1p.tile([128, KD * 128], f16, tag="w1")
                        nc.sync.dma_start(w1t_first[:], w1[0])
                yacc = yp.tile([128, ND * TT], f32, tag="yacc")

                for fb in range(NFB):
                    htile = hp.tile([128, KF * TT], f16, tag="h")
                    # ---- phase 1: hT[fb] = gelu(w1T.T @ xT + b1) ----
                    for fc in range(KF):
                        g = fb * KF + fc
                        if w1t_first is not None and fb == 0 and fc == 0:
                            w1t = w1t_first
                            w1t_first = None
                        else:
                            w1t = w1p.tile([128, KD * 128], f16, tag="w1")
                            nc.sync.dma_start(w1t[:], w1[g])
                        ph = hpp.tile([128, TT], f32, tag="hps")
                        for k in range(KD):
                            for th in range(TT // 512):
                                nc.tensor.matmul(
                                    ph[:, th * 512:(th + 1) * 512],
                                    w1t[:, k * 128:(k + 1) * 128],
                                    xks[k][:, th * 512:(th + 1) * 512],
                                    start=(k == 0),
                                    stop=(k == KD - 1),
                                )
                        nc.scalar.activation(
                            htile[:, fc * TT:(fc + 1) * TT], ph[:],
                            AFT.Gelu, bias=b1_sb[:, g:g + 1],
                        )

                    # ---- phase 2: yT += w2T.T @ hT[fb] (+ b2 on first block) ----
                    for dcg in range(ND // 2):
                        w2ts = []
                        for j in range(2):
                            dc = dcg * 2 + j
                            w2t = w2p.tile([128, KF * 128], f16, tag="w2")
                            nc.sync.dma_start(w2t[:], w2[fb, dc])
                            w2ts.append(w2t)
                        pys = [
                            ypp.tile([128, 512], f32, tag="yps", name=f"yps_{i}")
                            for i in range(2 * (TT // 512))
                        ]
                        for fc in range(KF):
                            for j in range(2):
                                for th in range(TT // 512):
                                    nc.tensor.matmul(
                                        pys[j * (TT // 512) + th][:],
                                        w2ts[j][:, fc * 128:(fc + 1) * 128],
                                        htile[:, fc * TT + th * 512:fc * TT + (th + 1) * 512],
                                        start=(fc == 0),
                                        stop=(fc == KF - 1),
                                    )  # j-major keeps w2 stationary across th

                        for j in range(2):
                            dc = dcg * 2 + j
                            yfs = []
                            for th in range(TT // 512):
                                py = pys[j * (TT // 512) + th][:]
                                yc = yacc[:, dc * TT + th * 512:dc * TT + (th + 1) * 512]
                                if fb == 0:
                                    nc.scalar.activation(
                                        yc, py, AFT.Identity, bias=b2_sb[:, dc:dc + 1]
                                    )
                                elif fb < NFB - 1:
                                    nc.vector.tensor_add(yc, yc, py)
                                else:
                                    yf = yfp.tile([128, 512], f16, tag="yf16")
                                    nc.vector.tensor_add(yf[:], yc, py)
                                    yfs.append(yf)
                            if fb == NFB - 1:
                                # store the finished d-chunk (fp16, half the
                                # bytes of the old fp32 path) while the
                                # remaining dcg compute still runs
                                for th, yf in enumerate(yfs):
                                    nc.scalar.dma_start(
                                        yt[dc * 128:(dc + 1) * 128,
                                           tt * TT + th * 512:tt * TT + (th + 1) * 512],
                                        yf[:],
                                    )

    nc.compile()
    return nc


def _get_compiled():
    global _COMPILED
    if _COMPILED is None:
        _COMPILED = _build()
    return _COMPILED


def _pack_core(x_e, w1_e, b1_e, w2_e, b2_e):
    """Host-side repack of one expert's tensors into the kernel's tiled layouts."""
    xT = x_e.reshape(T, D).T                      # [D, T]
    xt = np.ascontiguousarray(
        xT.reshape(KD, 128, NTT, TT).transpose(2, 0, 1, 3)
    ).astype(np.float16)                          # [NTT, KD, 128, TT]
    w1T = w1_e.T                                  # [D, F]
    w1t = np.ascontiguousarray(
        w1T.reshape(KD, 128, F // 128, 128).transpose(2, 1, 0, 3).reshape(F // 128, 128, KD * 128)
    ).astype(np.float16)                          # [F//128, 128, KD*128]
    w2T = w2_e.T                                  # [F, D]
    w2t = np.ascontiguousarray(
        w2T.reshape(NFB, KF, 128, ND, 128).transpose(0, 3, 2, 1, 4).reshape(NFB, ND, 128, KF * 128)
    ).astype(np.float16)                          # [NFB, ND, 128, KF*128]
    b1t = np.ascontiguousarray(b1_e.reshape(F // 128, 128).T)  # [128, F//128]
    b2t = np.ascontiguousarray(b2_e.reshape(ND, 128).T)        # [128, ND]
    return {"xt": xt, "w1": w1t, "w2": w2t, "b1": b1t, "b2": b2t}


def kernel(inputs, w1, b1, w2, b2):
    inputs = np.asarray(inputs, dtype=np.float32)
    w1 = np.asarray(w1, dtype=np.float32)
    b1 = np.asarray(b1, dtype=np.float32)
    w2 = np.asarray(w2, dtype=np.float32)
    b2 = np.asarray(b2, dtype=np.float32)

    nc = _get_compiled()

    in_maps = []
    for e in range(E):
        x_e = inputs[:, e * C:(e + 1) * C, :]     # [B, C, D]
        in_maps.append(_pack_core(x_e, w1[e], b1[e], w2[e], b2[e]))

    # The axon-tunneled devices occasionally come up wedged
    # (NRT_EXEC_UNIT_UNRECOVERABLE on the first execute); a retry after a
    # short pause reliably recovers.
    last_err = None
    for attempt in range(3):
        try:
            res = run_bass_kernel_spmd(nc, in_maps, core_ids=list(range(E)))
            out = np.empty((B, E * C, D), dtype=np.float32)
            for e in range(E):
                yT = np.asarray(res.results[e]["yt"]).astype(np.float32)  # [D, T]
                out[:, e * C:(e + 1) * C, :] = yT.T.reshape(B, C, D)
            return out
        except Exception as err:  # noqa: BLE001 - device flake, retry
            last_err = err
            time.sleep(10 * (attempt + 1))
    raise last_err
